# revision 45
# baseline (speedup 1.0000x reference)
"""Causal self-attention (B=4, T=1024, C=1024, H=16) on 8 TRN2 NeuronCores.

Sharding: tensor-parallel over heads — 2 heads per core. x is replicated;
each core computes qkv for its heads, attention, and a partial output
projection (its heads' columns of w_out); the host sums the 8 partials.

v2: all-bf16 matmul pipeline + HAM-aware scheduling.
  - All matmul operands are bf16 (PSUM accumulation stays fp32): 1 cyc/row
    at every moving width (fp32r pays 4 cyc/row below 256), halved SBUF
    traffic, and DVE 2x/4x perf modes on the elementwise ops.
  - qkv projection of batch b+1 is interleaved as PE "filler" work into
    the attention phase of batch b, so the PE never idles and the HAM
    clock gate stays at 2.4 GHz (idle gaps re-throttle it to 1.2 GHz).
  - x/w DMAs are chunked per contraction block with per-chunk tiles so the
    first matmul starts ~1us in instead of waiting on whole-tile DMAs.
  - copies are spread across engines: q/k PSUM->SBUF on ACT, v/mask/
    normalize on DVE, proj PSUM->SBUF on GpSimd (otherwise idle).
  - output partials are bf16 (halves output DMA); host sums in fp64.

Per-core dataflow:
  phase 1: qkvT[chan, tok] = wqkvT.T @ xT per 512-token group;
           q,k kept as [d, tok] bf16 (2 heads packed on 128 partitions);
           v PE-transposed to [tok, d] bf16 with a ones column appended.
  phase 2: per (batch, 512-query group, 128-key block):
           ST[key, query] = kT.T @ qT  (2 heads row-packed)
           PT = exp(ST/8) -> bf16  (causal: 512-block skipping; diagonal
           128x128 block gets a multiplicative 0/1 mask on PT)
           OT[d+1, query] += v_aug.T @ PT  (row 64 accumulates the softmax
           denominator via the ones column)
           normalize: y = OT[0:64] * broadcast(1/OT[64]) -> bf16
  phase 3: out[tok, :] = yT.T @ woT, PSUM copied to SBUF bf16, DMA to HBM.
"""

import sys
import types

import numpy as np

import concourse.bacc as bacc
import concourse.mybir as mybir
import concourse.tile as tile
from concourse.bass_utils import run_bass_kernel_spmd
from concourse.masks import make_identity

F32 = mybir.dt.float32
BF16 = mybir.dt.bfloat16
Exp = mybir.ActivationFunctionType.Exp

P = 128
B = 4
T = 1024
C = 1024
N_HEAD = 16
DH = 64
BT = B * T           # 4096 tokens
NCO = C // P         # 8 contraction blocks
NTG = BT // 512      # 8 token groups of 512
QG_PER_B = T // 512  # 2 query groups per batch
N_CORES = 8
H_LOC = N_HEAD // N_CORES  # 2 local heads

SCALE = 1.0 / np.sqrt(np.float32(DH))  # 0.125


def build_nc():
    nc = bacc.Bacc("TRN2", target_bir_lowering=False, debug=False)

    xT = nc.dram_tensor("xT", [C, BT], BF16, kind="ExternalInput")
    wq = nc.dram_tensor("wq", [C, 3 * P], BF16, kind="ExternalInput")
    wo = nc.dram_tensor("wo", [P, C], BF16, kind="ExternalInput")
    out = nc.dram_tensor("out", [BT, C], BF16, kind="ExternalOutput")

    with tile.TileContext(nc) as tc:
        with (
            tc.tile_pool(name="consts", bufs=1) as consts,
            tc.tile_pool(name="vt", bufs=3) as vtp,
            tc.tile_pool(name="pt", bufs=6) as ptp,
            tc.tile_pool(name="ep", bufs=4) as epp,
            tc.tile_pool(name="outp", bufs=4) as outp,
            tc.tile_pool(name="ps_mm", bufs=3, space="PSUM") as ps_mm,
            tc.tile_pool(name="ps_st", bufs=3, space="PSUM") as ps_st,
            tc.tile_pool(name="ps_ot", bufs=2, space="PSUM") as ps_ot,
        ):
            # ---- DMA: per-chunk tiles so the first matmul's deps are just
            # its own two chunk loads. tg0 chunks paired with wq chunks
            # first, then tg1, wo, then the rest.
            # tg0 is chunked per contraction block, issued alternating with
            # wq chunks on SP — the serial ~600ns descriptor-gens naturally
            # stage the arrivals just ahead of the PE's consumption.
            x0_sb = [
                consts.tile([P, 512], BF16, name=f"x0_{co}") for co in range(NCO)
            ]
            xg_sb = [None] + [
                consts.tile([P, NCO, 512], BF16, name=f"xg{tg}")
                for tg in range(1, NTG)
            ]
            wq_sb = [
                consts.tile([P, 3 * P], BF16, name=f"wq{co}") for co in range(NCO)
            ]
            wo_sb = consts.tile([P, C], BF16)

            def x_chunk(tg, co):
                if tg == 0:
                    return x0_sb[co][:]
                return xg_sb[tg][:, co, :]

            def wq_chunk(co):
                return wq_sb[co][:]

            xT_r = xT[:].rearrange("(a p) t -> p a t", p=P)
            for co in range(NCO):
                nc.sync.dma_start(
                    x0_sb[co][:], xT[co * P:(co + 1) * P, 0:512]
                )
                nc.sync.dma_start(wq_sb[co][:], wq[co * P:(co + 1) * P, :])
            nc.sync.dma_start(xg_sb[1][:], xT_r[:, :, 512:1024])
            nc.sync.dma_start(wo_sb[:], wo[:])

            qT_all = consts.tile([P, NTG, 512], BF16)
            kT_all = consts.tile([P, NTG, 512], BF16)
            # both heads' v in one tile: per key block kb the 130 columns are
            # [h0 chans (64) | ones | h1 chans (64) | ones]; head h's PV lhsT
            # is the contiguous 65-column slice starting at h*65. The ones
            # columns accumulate the softmax denominator.
            v_aug = consts.tile([P, BT // P, 2 * (DH + 1)], BF16, name="v")

            ident = consts.tile([P, P], BF16)
            make_identity(nc, ident[:])
            # multiplicative causal mask for the diagonal 128x128 block of
            # PT[key, query]: keep where col - row >= 0. Built in f32
            # (affine_select proven there), then cast to bf16.
            tri_f32 = consts.tile([P, P], F32)
            nc.gpsimd.memset(tri_f32[:], 1.0)
            nc.gpsimd.affine_select(
                out=tri_f32[:], in_=tri_f32[:],
                compare_op=mybir.AluOpType.is_ge, fill=0.0,
                base=0, pattern=[[1, P]], channel_multiplier=-1,
            )
            tri01 = consts.tile([P, P], BF16)
            nc.vector.tensor_copy(tri01[:], tri_f32[:])
            ones_f = consts.tile([P, BT // P], BF16)
            nc.vector.memset(ones_f[:], 1.0)
            for h in range(H_LOC):
                nc.vector.tensor_copy(
                    v_aug[:, :, h * (DH + 1) + DH], ones_f[:]
                )

            # ---- qkv projection emission units ----
            # one unit = one (tg, cb) PSUM accumulation group (8 matmuls)
            # plus its PSUM->SBUF copy; v additionally spawns 4 transpose
            # units. Units are either emitted directly (prologue) or pushed
            # through the filler queue during attention.

            def transpose_unit(tg, j, vt):
                kb = tg * 4 + j
                pst = ps_mm.tile([P, P], BF16, tag="mm", name="pst")
                nc.tensor.transpose(pst[:], vt[:, j * P:(j + 1) * P], ident[:])
                # both heads in one strided copy: dst groups at 0 and 65
                nc.vector.tensor_copy(
                    v_aug[:, kb, :].rearrange(
                        "p (g c) -> p g c", g=H_LOC
                    )[:, :, 0:DH],
                    pst[:, :].rearrange("p (g c) -> p g c", g=H_LOC),
                )

            def qkv_unit(tg, cb):
                # returns list of follow-up units (v transposes)
                ps = ps_mm.tile([P, 512], F32, tag="mm")
                for co in range(NCO):
                    nc.tensor.matmul(
                        ps[:],
                        lhsT=wq_chunk(co)[:, cb * P:(cb + 1) * P],
                        rhs=x_chunk(tg, co),
                        start=(co == 0), stop=(co == NCO - 1),
                    )
                if cb == 0:
                    nc.scalar.copy(qT_all[:, tg, :], ps[:])
                    return []
                if cb == 1:
                    nc.scalar.copy(kT_all[:, tg, :], ps[:])
                    return []
                vt = vtp.tile([P, 512], BF16, tag="vt")
                nc.scalar.copy(vt[:], ps[:])
                return [lambda j=j, vt=vt: transpose_unit(tg, j, vt)
                        for j in range(4)]

            def emit_qkv_group(tg):
                # prologue path: emit everything for one token group now
                followups = []
                for cb in range(3):
                    followups += qkv_unit(tg, cb)
                for f in followups:
                    f()

            # ---- prologue: qkv for tg0, tg1 (feeds attention for b=0) ----
            # later x groups are DMA'd only after the prologue's own loads
            # are in the queues, so they don't starve the first matmuls
            emit_qkv_group(0)
            for tg in (2, 3, 4):
                nc.sync.dma_start(
                    xg_sb[tg][:], xT_r[:, :, tg * 512:(tg + 1) * 512]
                )
            emit_qkv_group(1)
            for tg in (5, 6, 7):
                nc.sync.dma_start(
                    xg_sb[tg][:], xT_r[:, :, tg * 512:(tg + 1) * 512]
                )

            # ---- attention + projection, software-pipelined ----
            # one proj unit covers a full 128-token tile: 2 matmuls into the
            # two 512-column halves, copies to one SBUF tile, one DMA out
            def proj_step(tgq, yT, ti):
                tok0 = tgq * 512 + ti * P
                ob = outp.tile([P, C], BF16, tag="ob", name="ob")
                for cohalf in range(2):
                    po = ps_mm.tile([P, 512], F32, tag="mm", name="po")
                    nc.tensor.matmul(
                        po[:],
                        lhsT=yT[:, ti * P:(ti + 1) * P],
                        rhs=wo_sb[:, cohalf * 512:(cohalf + 1) * 512],
                        start=True, stop=True,
                    )
                    dst = ob[:, cohalf * 512:(cohalf + 1) * 512]
                    if cohalf == 0:
                        nc.vector.tensor_copy(dst, po[:])
                    else:
                        nc.scalar.copy(dst, po[:])
                nc.sync.dma_start(out[tok0:tok0 + P, :], ob[:])

            # filler queue of (kind, fn): independent PE work units — proj
            # steps of the previous group, qkv units + v transposes of the
            # next batch. Popped between attention iterations so the PE
            # never idles; qkv/transpose units are force-drained at batch
            # end (the next batch's attention consumes their outputs), proj
            # units can linger.
            filler = []

            def pop_filler(n=1, reserve=0):
                for _ in range(n):
                    if len(filler) <= reserve:
                        return
                    kind, fn = filler.pop(0)
                    more = fn()
                    if more:
                        filler[0:0] = [("tr", m) for m in more]

            def drain_kinds(kinds):
                rest = []
                while filler:
                    kind, fn = filler.pop(0)
                    if kind in kinds:
                        more = fn()
                        if more:
                            filler[0:0] = [("tr", m) for m in more]
                    else:
                        rest.append((kind, fn))
                filler.extend(rest)

            for b in range(B):
                # qkv work for batch b+1, to be drained during batch b's
                # attention
                if b < B - 1:
                    for tg in (2 * b + 2, 2 * b + 3):
                        for cb in range(3):
                            filler.append(
                                ("qkv", lambda tg=tg, cb=cb: qkv_unit(tg, cb))
                            )

                for qg in range(QG_PER_B):
                    tgq = QG_PER_B * b + qg
                    nkj_total = (qg + 1) * 4
                    OT = [
                        ps_ot.tile([DH + 1, 512], F32, tag="ot", name=f"ot{_h}")
                        for _h in range(H_LOC)
                    ]
                    pending = []  # [(pts, kb, q_lo, idx)] PV delayed 2 blocks

                    def emit_pv(batch):
                        for h in range(H_LOC):
                            for pts, kb_, q_lo_, idx_ in batch:
                                nc.tensor.matmul(
                                    OT[h][:, q_lo_:512],
                                    lhsT=v_aug[
                                        :, kb_, h * (DH + 1):(h + 1) * (DH + 1)
                                    ],
                                    rhs=pts[h][:, q_lo_:512],
                                    start=(idx_ == 0),
                                    stop=(idx_ == nkj_total - 1),
                                )

                    idx = 0
                    for kg in range(qg + 1):
                        diag = kg == qg
                        tgk = QG_PER_B * b + kg
                        for kj in range(4):
                            kb = tgk * 4 + kj
                            q_lo = kj * P if diag else 0
                            pts = []
                            for h in range(H_LOC):
                                hs = slice(h * DH, (h + 1) * DH)
                                st = ps_st.tile(
                                    [P, 512], F32, tag="st", name="st"
                                )
                                nc.tensor.matmul(
                                    st[:, q_lo:512],
                                    lhsT=kT_all[hs, tgk, kj * P:(kj + 1) * P],
                                    rhs=qT_all[hs, tgq, q_lo:512],
                                    start=True, stop=True,
                                )
                                pt = ptp.tile([P, 512], BF16, tag="pt", name="pt")
                                nc.scalar.activation(
                                    pt[:, q_lo:512], st[:, q_lo:512], Exp,
                                    bias=0.0, scale=float(SCALE),
                                )
                                if diag:
                                    nc.vector.tensor_mul(
                                        pt[:, kj * P:(kj + 1) * P],
                                        pt[:, kj * P:(kj + 1) * P],
                                        tri01[:],
                                    )
                                pts.append(pt)
                            pending.append((pts, kb, q_lo, idx))
                            idx += 1
                            if len(pending) == 2 and idx < nkj_total:
                                emit_pv(pending)
                                pending = []
                            if idx >= 1:
                                # adaptive pacing: drain fast only when the
                                # queue is deep, so late batches (no qkv
                                # work left) still have proj fillers
                                pop_filler(2 if len(filler) > 9 else 1)
                    emit_pv(pending)
                    if qg == QG_PER_B - 1:
                        # next batch's attention needs qT/kT/v complete
                        drain_kinds(("qkv", "tr"))

                    # epilogue: normalize y = O / den (den in OT row 64)
                    last_group = (b == B - 1) and (qg == QG_PER_B - 1)
                    yT = epp.tile([P, 512], BF16, tag="yt")
                    rbs = []
                    for h in range(H_LOC):
                        den = epp.tile([1, 512], F32, tag="den", name="den")
                        nc.vector.tensor_copy(den[:], OT[h][DH:DH + 1, :])
                        rbr = epp.tile([DH, 512], F32, tag="rbr", name="rbr")
                        nc.gpsimd.partition_broadcast(rbr[:], den[:], channels=DH)
                        rb = epp.tile([DH, 512], F32, tag="rb", name="rb")
                        nc.vector.reciprocal_approx_fast(rb[:], rbr[:])
                        rbs.append(rb)
                        if not last_group:
                            nc.vector.tensor_mul(
                                yT[h * DH:(h + 1) * DH, :], OT[h][0:DH, :],
                                rb[:],
                            )
                    if not last_group:
                        filler += [
                            ("proj",
                             lambda tgq=tgq, yT=yT, ti=ti:
                             proj_step(tgq, yT, ti))
                            for ti in range(4)
                        ]
                    else:
                        # tail: chunk the normalize per token tile and chase
                        # each chunk with its projection matmuls immediately
                        for ti in range(4):
                            ts_ = slice(ti * P, (ti + 1) * P)
                            for h in range(H_LOC):
                                nc.vector.tensor_mul(
                                    yT[h * DH:(h + 1) * DH, ts_],
                                    OT[h][0:DH, ts_], rbs[h][:, ts_],
                                )
                            proj_step(tgq, yT, ti)

            drain_kinds(("qkv", "tr", "proj"))

    nc.compile()
    return nc


def _install_ntff_hook():
    try:
        from antenv.axon_hooks import get_axon_ntff_profile_hook  # noqa: F401
        return
    except ImportError:
        pass
    try:
        import trn_agent_boot.trn_boot as tb
        hook = tb._ntff_profile_via_ctypes("/opt/axon/libaxon_pjrt.so")
        mod = types.ModuleType("antenv.axon_hooks")
        mod.get_axon_ntff_profile_hook = lambda: hook
        mod.set_axon_ntff_profile_hook = lambda h: None
        sys.modules["antenv.axon_hooks"] = mod
    except Exception:
        pass


_NC_CACHE = None
LAST_EXEC_NS = None
LAST_TRACE = None


def kernel(x, w_qkv, w_out, trace=False):
    global _NC_CACHE, LAST_EXEC_NS, LAST_TRACE
    import ml_dtypes
    bf16 = ml_dtypes.bfloat16

    if _NC_CACHE is None:
        _NC_CACHE = build_nc()
    nc = _NC_CACHE

    x = np.asarray(x, dtype=np.float32)
    w_qkv = np.asarray(w_qkv, dtype=np.float32)
    w_out = np.asarray(w_out, dtype=np.float32)

    xT = np.ascontiguousarray(x.reshape(BT, C).T.astype(bf16))  # [C, BT]
    in_maps = []
    for core in range(N_CORES):
        h0 = core * H_LOC * DH  # first local channel
        ch = slice(h0, h0 + H_LOC * DH)
        # wq columns: [q chans | k chans | v chans] for the local heads
        wq_i = np.concatenate(
            [w_qkv[s * C:s * C + C, :][ch, :] for s in range(3)], axis=0
        )  # [384, C]
        in_maps.append({
            "xT": xT,
            "wq": np.ascontiguousarray(wq_i.T.astype(bf16)),        # [C, 384]
            "wo": np.ascontiguousarray(w_out[:, ch].T.astype(bf16)),  # [128, C]
        })

    if trace:
        _install_ntff_hook()
    res = run_bass_kernel_spmd(
        nc, in_maps, core_ids=list(range(N_CORES)), trace=trace
    )
    LAST_EXEC_NS = res.exec_time_ns
    kernel_globals = globals()
    kernel_globals['LAST_RESULT'] = res
    LAST_TRACE = (
        res.instructions_and_trace[1] if res.instructions_and_trace else None
    )

    acc = np.zeros((BT, C), dtype=np.float64)
    for core in range(N_CORES):
        acc += res.results[core]["out"].astype(np.float64)
    return acc.astype(np.float32).reshape(B, T, C)


# revision 48
# speedup vs baseline: 1.1854x; 1.1854x over previous
"""Causal self-attention (B=4, T=1024, C=1024, H=16) on 8 TRN2 NeuronCores.

Sharding: tensor-parallel over heads — 2 heads per core. x is replicated;
each core computes qkv for its heads, attention, and a partial output
projection (its heads' columns of w_out); the host sums the 8 partials.

v2: all-bf16 matmul pipeline + HAM-aware scheduling.
  - All matmul operands are bf16 (PSUM accumulation stays fp32): 1 cyc/row
    at every moving width (fp32r pays 4 cyc/row below 256), halved SBUF
    traffic, and DVE 2x/4x perf modes on the elementwise ops.
  - qkv projection of batch b+1 is interleaved as PE "filler" work into
    the attention phase of batch b, so the PE never idles and the HAM
    clock gate stays at 2.4 GHz (idle gaps re-throttle it to 1.2 GHz).
  - x/w DMAs are chunked per contraction block with per-chunk tiles so the
    first matmul starts ~1us in instead of waiting on whole-tile DMAs.
  - copies are spread across engines: q/k PSUM->SBUF on ACT, v/mask/
    normalize on DVE, proj PSUM->SBUF on GpSimd (otherwise idle).
  - output partials are bf16 (halves output DMA); host sums in fp64.

Per-core dataflow:
  phase 1: qkvT[chan, tok] = wqkvT.T @ xT per 512-token group;
           q,k kept as [d, tok] bf16 (2 heads packed on 128 partitions);
           v PE-transposed to [tok, d] bf16 with a ones column appended.
  phase 2: per (batch, 512-query group, 128-key block):
           ST[key, query] = kT.T @ qT  (2 heads row-packed)
           PT = exp(ST/8) -> bf16  (causal: 512-block skipping; diagonal
           128x128 block gets a multiplicative 0/1 mask on PT)
           OT[d+1, query] += v_aug.T @ PT  (row 64 accumulates the softmax
           denominator via the ones column)
           normalize: y = OT[0:64] * broadcast(1/OT[64]) -> bf16
  phase 3: out[tok, :] = yT.T @ woT, PSUM copied to SBUF bf16, DMA to HBM.
"""

import sys
import types

import numpy as np

import concourse.bacc as bacc
import concourse.mybir as mybir
import concourse.tile as tile
from concourse.bass_utils import run_bass_kernel_spmd
from concourse.masks import make_identity

F32 = mybir.dt.float32
BF16 = mybir.dt.bfloat16
Exp = mybir.ActivationFunctionType.Exp

P = 128
B = 4
T = 1024
C = 1024
N_HEAD = 16
DH = 64
BT = B * T           # 4096 tokens
NCO = C // P         # 8 contraction blocks
NTG = BT // 512      # 8 token groups of 512
QG_PER_B = T // 512  # 2 query groups per batch
N_CORES = 8
H_LOC = N_HEAD // N_CORES  # 2 local heads

SCALE = 1.0 / np.sqrt(np.float32(DH))  # 0.125


def build_nc():
    nc = bacc.Bacc("TRN2", target_bir_lowering=False, debug=False)

    xT = nc.dram_tensor("xT", [C, BT], BF16, kind="ExternalInput")
    wq = nc.dram_tensor("wq", [C, 3 * P], BF16, kind="ExternalInput")
    wo = nc.dram_tensor("wo", [P, C], BF16, kind="ExternalInput")
    out = nc.dram_tensor("out", [BT, C], BF16, kind="ExternalOutput")

    with tile.TileContext(nc) as tc:
        with (
            tc.tile_pool(name="consts", bufs=1) as consts,
            tc.tile_pool(name="vt", bufs=3) as vtp,
            tc.tile_pool(name="pt", bufs=6) as ptp,
            tc.tile_pool(name="ep", bufs=4) as epp,
            tc.tile_pool(name="outp", bufs=4) as outp,
            tc.tile_pool(name="ps_mm", bufs=3, space="PSUM") as ps_mm,
            tc.tile_pool(name="ps_st", bufs=3, space="PSUM") as ps_st,
            tc.tile_pool(name="ps_ot", bufs=2, space="PSUM") as ps_ot,
        ):
            # ---- DMA: per-chunk tiles so the first matmul's deps are just
            # its own two chunk loads. tg0 chunks paired with wq chunks
            # first, then tg1, wo, then the rest.
            # tg0 is chunked per contraction block, issued alternating with
            # wq chunks on SP — the serial ~600ns descriptor-gens naturally
            # stage the arrivals just ahead of the PE's consumption.
            x0_sb = [
                consts.tile([P, 512], BF16, name=f"x0_{co}") for co in range(NCO)
            ]
            xg_sb = [None] + [
                consts.tile([P, NCO, 512], BF16, name=f"xg{tg}")
                for tg in range(1, NTG)
            ]
            wq_sb = [
                consts.tile([P, 3 * P], BF16, name=f"wq{co}") for co in range(NCO)
            ]
            wo_sb = consts.tile([P, C], BF16)

            def x_chunk(tg, co):
                if tg == 0:
                    return x0_sb[co][:]
                return xg_sb[tg][:, co, :]

            def wq_chunk(co):
                return wq_sb[co][:]

            xT_r = xT[:].rearrange("(a p) t -> p a t", p=P)
            for co in range(NCO):
                nc.sync.dma_start(
                    x0_sb[co][:], xT[co * P:(co + 1) * P, 0:512]
                )
                nc.sync.dma_start(wq_sb[co][:], wq[co * P:(co + 1) * P, :])
            nc.sync.dma_start(xg_sb[1][:], xT_r[:, :, 512:1024])
            nc.sync.dma_start(wo_sb[:], wo[:])

            qT_all = consts.tile([P, NTG, 512], BF16)
            kT_all = consts.tile([P, NTG, 512], BF16)
            # both heads' v in one tile: per key block kb the 130 columns are
            # [h0 chans (64) | ones | h1 chans (64) | ones]; head h's PV lhsT
            # is the contiguous 65-column slice starting at h*65. The ones
            # columns accumulate the softmax denominator.
            v_aug = consts.tile([P, BT // P, 2 * (DH + 1)], BF16, name="v")

            ident = consts.tile([P, P], F32)
            make_identity(nc, ident[:])
            # multiplicative causal mask for the diagonal 128x128 block of
            # PT[key, query]: keep where col - row >= 0. Built in f32
            # (affine_select proven there), then cast to bf16.
            tri_f32 = consts.tile([P, P], F32)
            nc.gpsimd.memset(tri_f32[:], 1.0)
            nc.gpsimd.affine_select(
                out=tri_f32[:], in_=tri_f32[:],
                compare_op=mybir.AluOpType.is_ge, fill=0.0,
                base=0, pattern=[[1, P]], channel_multiplier=-1,
            )
            tri01 = consts.tile([P, P], BF16)
            nc.vector.tensor_copy(tri01[:], tri_f32[:])
            ones_f = consts.tile([P, BT // P], BF16)
            nc.vector.memset(ones_f[:], 1.0)
            for h in range(H_LOC):
                nc.vector.tensor_copy(
                    v_aug[:, :, h * (DH + 1) + DH], ones_f[:]
                )

            # ---- qkv projection emission units ----
            # one unit = one (tg, cb) PSUM accumulation group (8 matmuls)
            # plus its PSUM->SBUF copy; v additionally spawns 4 transpose
            # units. Units are either emitted directly (prologue) or pushed
            # through the filler queue during attention.

            def transpose_unit(tg, j, vt):
                kb = tg * 4 + j
                pst = ps_mm.tile([P, P], F32, tag="mm", name="pst")
                nc.tensor.transpose(pst[:], vt[:, j * P:(j + 1) * P], ident[:])
                # both heads in one strided copy: dst groups at 0 and 65
                nc.vector.tensor_copy(
                    v_aug[:, kb, :].rearrange(
                        "p (g c) -> p g c", g=H_LOC
                    )[:, :, 0:DH],
                    pst[:, :].rearrange("p (g c) -> p g c", g=H_LOC),
                )

            def qkv_unit(tg, cb):
                # returns list of follow-up units (v transposes)
                ps = ps_mm.tile([P, 512], F32, tag="mm")
                for co in range(NCO):
                    nc.tensor.matmul(
                        ps[:],
                        lhsT=wq_chunk(co)[:, cb * P:(cb + 1) * P],
                        rhs=x_chunk(tg, co),
                        start=(co == 0), stop=(co == NCO - 1),
                    )
                if cb == 0:
                    nc.scalar.copy(qT_all[:, tg, :], ps[:])
                    return []
                if cb == 1:
                    nc.scalar.copy(kT_all[:, tg, :], ps[:])
                    return []
                vt = vtp.tile([P, 512], F32, tag="vt")
                nc.scalar.copy(vt[:], ps[:])
                return [lambda j=j, vt=vt: transpose_unit(tg, j, vt)
                        for j in range(4)]

            def emit_qkv_group(tg):
                # prologue path: emit everything for one token group now
                followups = []
                for cb in range(3):
                    followups += qkv_unit(tg, cb)
                for f in followups:
                    f()

            # ---- prologue: qkv for tg0, tg1 (feeds attention for b=0) ----
            # later x groups are DMA'd only after the prologue's own loads
            # are in the queues, so they don't starve the first matmuls
            emit_qkv_group(0)
            for tg in (2, 3, 4):
                nc.sync.dma_start(
                    xg_sb[tg][:], xT_r[:, :, tg * 512:(tg + 1) * 512]
                )
            emit_qkv_group(1)
            for tg in (5, 6, 7):
                nc.sync.dma_start(
                    xg_sb[tg][:], xT_r[:, :, tg * 512:(tg + 1) * 512]
                )

            # ---- attention + projection, software-pipelined ----
            # one proj unit covers a full 128-token tile: 2 matmuls into the
            # two 512-column halves, copies to one SBUF tile, one DMA out
            def proj_step(tgq, yT, ti):
                tok0 = tgq * 512 + ti * P
                ob = outp.tile([P, C], BF16, tag="ob", name="ob")
                for cohalf in range(2):
                    po = ps_mm.tile([P, 512], F32, tag="mm", name="po")
                    nc.tensor.matmul(
                        po[:],
                        lhsT=yT[:, ti * P:(ti + 1) * P],
                        rhs=wo_sb[:, cohalf * 512:(cohalf + 1) * 512],
                        start=True, stop=True,
                    )
                    dst = ob[:, cohalf * 512:(cohalf + 1) * 512]
                    if cohalf == 0:
                        nc.vector.tensor_copy(dst, po[:])
                    else:
                        nc.scalar.copy(dst, po[:])
                nc.sync.dma_start(out[tok0:tok0 + P, :], ob[:])

            # filler queue of (kind, fn): independent PE work units — proj
            # steps of the previous group, qkv units + v transposes of the
            # next batch. Popped between attention iterations so the PE
            # never idles; qkv/transpose units are force-drained at batch
            # end (the next batch's attention consumes their outputs), proj
            # units can linger.
            filler = []

            def pop_filler(n=1, reserve=0):
                for _ in range(n):
                    if len(filler) <= reserve:
                        return
                    kind, fn = filler.pop(0)
                    more = fn()
                    if more:
                        filler[0:0] = [("tr", m) for m in more]

            def drain_kinds(kinds):
                rest = []
                while filler:
                    kind, fn = filler.pop(0)
                    if kind in kinds:
                        more = fn()
                        if more:
                            filler[0:0] = [("tr", m) for m in more]
                    else:
                        rest.append((kind, fn))
                filler.extend(rest)

            for b in range(B):
                # qkv work for batch b+1, to be drained during batch b's
                # attention
                if b < B - 1:
                    for tg in (2 * b + 2, 2 * b + 3):
                        for cb in range(3):
                            filler.append(
                                ("qkv", lambda tg=tg, cb=cb: qkv_unit(tg, cb))
                            )

                for qg in range(QG_PER_B):
                    tgq = QG_PER_B * b + qg
                    nkj_total = (qg + 1) * 4
                    OT = [
                        ps_ot.tile([DH + 1, 512], F32, tag="ot", name=f"ot{_h}")
                        for _h in range(H_LOC)
                    ]
                    pending = []  # [(pts, kb, q_lo, idx)] PV delayed 2 blocks

                    def emit_pv(batch):
                        for h in range(H_LOC):
                            for pts, kb_, q_lo_, idx_ in batch:
                                nc.tensor.matmul(
                                    OT[h][:, q_lo_:512],
                                    lhsT=v_aug[
                                        :, kb_, h * (DH + 1):(h + 1) * (DH + 1)
                                    ],
                                    rhs=pts[h][:, q_lo_:512],
                                    start=(idx_ == 0),
                                    stop=(idx_ == nkj_total - 1),
                                )

                    idx = 0
                    for kg in range(qg + 1):
                        diag = kg == qg
                        tgk = QG_PER_B * b + kg
                        for kj in range(4):
                            kb = tgk * 4 + kj
                            q_lo = kj * P if diag else 0
                            pts = []
                            for h in range(H_LOC):
                                hs = slice(h * DH, (h + 1) * DH)
                                st = ps_st.tile(
                                    [P, 512], F32, tag="st", name="st"
                                )
                                nc.tensor.matmul(
                                    st[:, q_lo:512],
                                    lhsT=kT_all[hs, tgk, kj * P:(kj + 1) * P],
                                    rhs=qT_all[hs, tgq, q_lo:512],
                                    start=True, stop=True,
                                )
                                pt = ptp.tile([P, 512], BF16, tag="pt", name="pt")
                                nc.scalar.activation(
                                    pt[:, q_lo:512], st[:, q_lo:512], Exp,
                                    bias=0.0, scale=float(SCALE),
                                )
                                if diag:
                                    nc.vector.tensor_mul(
                                        pt[:, kj * P:(kj + 1) * P],
                                        pt[:, kj * P:(kj + 1) * P],
                                        tri01[:],
                                    )
                                pts.append(pt)
                            pending.append((pts, kb, q_lo, idx))
                            idx += 1
                            if len(pending) == 2 and idx < nkj_total:
                                emit_pv(pending)
                                pending = []
                            if idx >= 1:
                                # adaptive pacing: drain fast only when the
                                # queue is deep, so late batches (no qkv
                                # work left) still have proj fillers
                                pop_filler(2 if len(filler) > 9 else 1)
                    emit_pv(pending)
                    if qg == QG_PER_B - 1:
                        # next batch's attention needs qT/kT/v complete
                        drain_kinds(("qkv", "tr"))

                    # epilogue: normalize y = O / den (den in OT row 64)
                    last_group = (b == B - 1) and (qg == QG_PER_B - 1)
                    yT = epp.tile([P, 512], BF16, tag="yt")
                    rbs = []
                    for h in range(H_LOC):
                        den = epp.tile([1, 512], F32, tag="den", name="den")
                        nc.vector.tensor_copy(den[:], OT[h][DH:DH + 1, :])
                        rbr = epp.tile([DH, 512], F32, tag="rbr", name="rbr")
                        nc.gpsimd.partition_broadcast(rbr[:], den[:], channels=DH)
                        rb = epp.tile([DH, 512], F32, tag="rb", name="rb")
                        nc.vector.reciprocal_approx_fast(rb[:], rbr[:])
                        rbs.append(rb)
                        if not last_group:
                            nc.vector.tensor_mul(
                                yT[h * DH:(h + 1) * DH, :], OT[h][0:DH, :],
                                rb[:],
                            )
                    if not last_group:
                        filler += [
                            ("proj",
                             lambda tgq=tgq, yT=yT, ti=ti:
                             proj_step(tgq, yT, ti))
                            for ti in range(4)
                        ]
                    else:
                        # tail: chunk the normalize per token tile and chase
                        # each chunk with its projection matmuls immediately
                        for ti in range(4):
                            ts_ = slice(ti * P, (ti + 1) * P)
                            for h in range(H_LOC):
                                nc.vector.tensor_mul(
                                    yT[h * DH:(h + 1) * DH, ts_],
                                    OT[h][0:DH, ts_], rbs[h][:, ts_],
                                )
                            proj_step(tgq, yT, ti)

            drain_kinds(("qkv", "tr", "proj"))

    nc.compile()
    return nc


def _install_ntff_hook():
    try:
        from antenv.axon_hooks import get_axon_ntff_profile_hook  # noqa: F401
        return
    except ImportError:
        pass
    try:
        import trn_agent_boot.trn_boot as tb
        hook = tb._ntff_profile_via_ctypes("/opt/axon/libaxon_pjrt.so")
        mod = types.ModuleType("antenv.axon_hooks")
        mod.get_axon_ntff_profile_hook = lambda: hook
        mod.set_axon_ntff_profile_hook = lambda h: None
        sys.modules["antenv.axon_hooks"] = mod
    except Exception:
        pass


_NC_CACHE = None
LAST_EXEC_NS = None
LAST_TRACE = None


def kernel(x, w_qkv, w_out, trace=False):
    global _NC_CACHE, LAST_EXEC_NS, LAST_TRACE
    import ml_dtypes
    bf16 = ml_dtypes.bfloat16

    if _NC_CACHE is None:
        _NC_CACHE = build_nc()
    nc = _NC_CACHE

    x = np.asarray(x, dtype=np.float32)
    w_qkv = np.asarray(w_qkv, dtype=np.float32)
    w_out = np.asarray(w_out, dtype=np.float32)

    xT = np.ascontiguousarray(x.reshape(BT, C).T.astype(bf16))  # [C, BT]
    in_maps = []
    for core in range(N_CORES):
        h0 = core * H_LOC * DH  # first local channel
        ch = slice(h0, h0 + H_LOC * DH)
        # wq columns: [q chans | k chans | v chans] for the local heads
        wq_i = np.concatenate(
            [w_qkv[s * C:s * C + C, :][ch, :] for s in range(3)], axis=0
        )  # [384, C]
        in_maps.append({
            "xT": xT,
            "wq": np.ascontiguousarray(wq_i.T.astype(bf16)),        # [C, 384]
            "wo": np.ascontiguousarray(w_out[:, ch].T.astype(bf16)),  # [128, C]
        })

    if trace:
        _install_ntff_hook()
    res = run_bass_kernel_spmd(
        nc, in_maps, core_ids=list(range(N_CORES)), trace=trace
    )
    LAST_EXEC_NS = res.exec_time_ns
    kernel_globals = globals()
    kernel_globals['LAST_RESULT'] = res
    LAST_TRACE = (
        res.instructions_and_trace[1] if res.instructions_and_trace else None
    )

    acc = np.zeros((BT, C), dtype=np.float64)
    for core in range(N_CORES):
        acc += res.results[core]["out"].astype(np.float64)
    return acc.astype(np.float32).reshape(B, T, C)


# revision 49
# speedup vs baseline: 1.1900x; 1.0039x over previous
"""Causal self-attention (B=4, T=1024, C=1024, H=16) on 8 TRN2 NeuronCores.

Sharding: tensor-parallel over heads — 2 heads per core. x is replicated;
each core computes qkv for its heads, attention, and a partial output
projection (its heads' columns of w_out); the host sums the 8 partials.

v2: all-bf16 matmul pipeline + HAM-aware scheduling.
  - All matmul operands are bf16 (PSUM accumulation stays fp32): 1 cyc/row
    at every moving width (fp32r pays 4 cyc/row below 256), halved SBUF
    traffic, and DVE 2x/4x perf modes on the elementwise ops.
  - qkv projection of batch b+1 is interleaved as PE "filler" work into
    the attention phase of batch b, so the PE never idles and the HAM
    clock gate stays at 2.4 GHz (idle gaps re-throttle it to 1.2 GHz).
  - x/w DMAs are chunked per contraction block with per-chunk tiles so the
    first matmul starts ~1us in instead of waiting on whole-tile DMAs.
  - copies are spread across engines: q/k PSUM->SBUF on ACT, v/mask/
    normalize on DVE, proj PSUM->SBUF on GpSimd (otherwise idle).
  - output partials are bf16 (halves output DMA); host sums in fp64.

Per-core dataflow:
  phase 1: qkvT[chan, tok] = wqkvT.T @ xT per 512-token group;
           q,k kept as [d, tok] bf16 (2 heads packed on 128 partitions);
           v PE-transposed to [tok, d] bf16 with a ones column appended.
  phase 2: per (batch, 512-query group, 128-key block):
           ST[key, query] = kT.T @ qT  (2 heads row-packed)
           PT = exp(ST/8) -> bf16  (causal: 512-block skipping; diagonal
           128x128 block gets a multiplicative 0/1 mask on PT)
           OT[d+1, query] += v_aug.T @ PT  (row 64 accumulates the softmax
           denominator via the ones column)
           normalize: y = OT[0:64] * broadcast(1/OT[64]) -> bf16
  phase 3: out[tok, :] = yT.T @ woT, PSUM copied to SBUF bf16, DMA to HBM.
"""

import sys
import types

import numpy as np

import concourse.bacc as bacc
import concourse.mybir as mybir
import concourse.tile as tile
from concourse.bass_utils import run_bass_kernel_spmd
from concourse.masks import make_identity

F32 = mybir.dt.float32
BF16 = mybir.dt.bfloat16
Exp = mybir.ActivationFunctionType.Exp

P = 128
B = 4
T = 1024
C = 1024
N_HEAD = 16
DH = 64
BT = B * T           # 4096 tokens
NCO = C // P         # 8 contraction blocks
NTG = BT // 512      # 8 token groups of 512
QG_PER_B = T // 512  # 2 query groups per batch
N_CORES = 8
H_LOC = N_HEAD // N_CORES  # 2 local heads

SCALE = 1.0 / np.sqrt(np.float32(DH))  # 0.125


def build_nc():
    nc = bacc.Bacc("TRN2", target_bir_lowering=False, debug=False)

    xT = nc.dram_tensor("xT", [C, BT], BF16, kind="ExternalInput")
    wq = nc.dram_tensor("wq", [C, 3 * P], BF16, kind="ExternalInput")
    wo = nc.dram_tensor("wo", [P, C], BF16, kind="ExternalInput")
    out = nc.dram_tensor("out", [BT, C], BF16, kind="ExternalOutput")

    with tile.TileContext(nc) as tc:
        with (
            tc.tile_pool(name="consts", bufs=1) as consts,
            tc.tile_pool(name="vt", bufs=3) as vtp,
            tc.tile_pool(name="pt", bufs=6) as ptp,
            tc.tile_pool(name="ep", bufs=4) as epp,
            tc.tile_pool(name="outp", bufs=4) as outp,
            tc.tile_pool(name="ps_mm", bufs=3, space="PSUM") as ps_mm,
            tc.tile_pool(name="ps_st", bufs=3, space="PSUM") as ps_st,
            tc.tile_pool(name="ps_ot", bufs=2, space="PSUM") as ps_ot,
        ):
            # ---- DMA: per-chunk tiles so the first matmul's deps are just
            # its own two chunk loads. tg0 chunks paired with wq chunks
            # first, then tg1, wo, then the rest.
            # tg0 is chunked per contraction block, issued alternating with
            # wq chunks on SP — the serial ~600ns descriptor-gens naturally
            # stage the arrivals just ahead of the PE's consumption.
            x0_sb = [
                consts.tile([P, 512], BF16, name=f"x0_{co}") for co in range(NCO)
            ]
            xg_sb = [None] + [
                consts.tile([P, NCO, 512], BF16, name=f"xg{tg}")
                for tg in range(1, NTG)
            ]
            wq_sb = [
                consts.tile([P, 3 * P], BF16, name=f"wq{co}") for co in range(NCO)
            ]
            wo_sb = consts.tile([P, C], BF16)

            def x_chunk(tg, co):
                if tg == 0:
                    return x0_sb[co][:]
                return xg_sb[tg][:, co, :]

            def wq_chunk(co):
                return wq_sb[co][:]

            xT_r = xT[:].rearrange("(a p) t -> p a t", p=P)
            for co in range(NCO):
                nc.sync.dma_start(
                    x0_sb[co][:], xT[co * P:(co + 1) * P, 0:512]
                )
                nc.sync.dma_start(wq_sb[co][:], wq[co * P:(co + 1) * P, :])
            nc.sync.dma_start(xg_sb[1][:], xT_r[:, :, 512:1024])
            nc.sync.dma_start(wo_sb[:], wo[:])

            qT_all = consts.tile([P, NTG, 512], BF16)
            kT_all = consts.tile([P, NTG, 512], BF16)
            # both heads' v in one tile: per key block kb the 130 columns are
            # [h0 chans (64) | ones | h1 chans (64) | ones]; head h's PV lhsT
            # is the contiguous 65-column slice starting at h*65. The ones
            # columns accumulate the softmax denominator.
            v_aug = consts.tile([P, BT // P, 2 * (DH + 1)], BF16, name="v")

            ident = consts.tile([P, P], F32)
            make_identity(nc, ident[:])
            # multiplicative causal mask for the diagonal 128x128 block of
            # PT[key, query]: keep where col - row >= 0. Built in f32
            # (affine_select proven there), then cast to bf16.
            tri_f32 = consts.tile([P, P], F32)
            nc.gpsimd.memset(tri_f32[:], 1.0)
            nc.gpsimd.affine_select(
                out=tri_f32[:], in_=tri_f32[:],
                compare_op=mybir.AluOpType.is_ge, fill=0.0,
                base=0, pattern=[[1, P]], channel_multiplier=-1,
            )
            tri01 = consts.tile([P, P], BF16)
            nc.vector.tensor_copy(tri01[:], tri_f32[:])
            ones_f = consts.tile([P, BT // P], BF16)
            nc.vector.memset(ones_f[:], 1.0)
            for h in range(H_LOC):
                nc.vector.tensor_copy(
                    v_aug[:, :, h * (DH + 1) + DH], ones_f[:]
                )

            # ---- qkv projection emission units ----
            # one unit = one (tg, cb) PSUM accumulation group (8 matmuls)
            # plus its PSUM->SBUF copy; v additionally spawns 4 transpose
            # units. Units are either emitted directly (prologue) or pushed
            # through the filler queue during attention.

            def transpose_unit(tg, j, vt):
                kb = tg * 4 + j
                pst = ps_mm.tile([P, P], F32, tag="mm", name="pst")
                nc.tensor.transpose(pst[:], vt[:, j * P:(j + 1) * P], ident[:])
                # both heads in one strided copy: dst groups at 0 and 65
                nc.vector.tensor_copy(
                    v_aug[:, kb, :].rearrange(
                        "p (g c) -> p g c", g=H_LOC
                    )[:, :, 0:DH],
                    pst[:, :].rearrange("p (g c) -> p g c", g=H_LOC),
                )

            def qkv_unit(tg, cb):
                # returns list of follow-up units (v transposes)
                ps = ps_mm.tile([P, 512], F32, tag="mm")
                for co in range(NCO):
                    nc.tensor.matmul(
                        ps[:],
                        lhsT=wq_chunk(co)[:, cb * P:(cb + 1) * P],
                        rhs=x_chunk(tg, co),
                        start=(co == 0), stop=(co == NCO - 1),
                    )
                if cb == 0:
                    nc.scalar.copy(qT_all[:, tg, :], ps[:])
                    return []
                if cb == 1:
                    nc.scalar.copy(kT_all[:, tg, :], ps[:])
                    return []
                vt = vtp.tile([P, 512], F32, tag="vt")
                nc.scalar.copy(vt[:], ps[:])
                return [lambda j=j, vt=vt: transpose_unit(tg, j, vt)
                        for j in range(4)]

            def emit_qkv_group(tg):
                # prologue path: emit everything for one token group now
                followups = []
                for cb in range(3):
                    followups += qkv_unit(tg, cb)
                for f in followups:
                    f()

            # ---- prologue: qkv for tg0, tg1 (feeds attention for b=0) ----
            # later x groups are DMA'd only after the prologue's own loads
            # are in the queues, so they don't starve the first matmuls
            emit_qkv_group(0)
            for tg in (2, 3, 4):
                nc.sync.dma_start(
                    xg_sb[tg][:], xT_r[:, :, tg * 512:(tg + 1) * 512]
                )
            emit_qkv_group(1)
            for tg in (5, 6, 7):
                nc.sync.dma_start(
                    xg_sb[tg][:], xT_r[:, :, tg * 512:(tg + 1) * 512]
                )

            # ---- attention + projection, software-pipelined ----
            # one proj unit covers a full 128-token tile: 2 matmuls into the
            # two 512-column halves, copies to one SBUF tile, one DMA out
            def proj_step(tgq, yT, ti):
                tok0 = tgq * 512 + ti * P
                ob = outp.tile([P, C], BF16, tag="ob", name="ob")
                for cohalf in range(2):
                    po = ps_mm.tile([P, 512], F32, tag="mm", name="po")
                    nc.tensor.matmul(
                        po[:],
                        lhsT=yT[:, ti * P:(ti + 1) * P],
                        rhs=wo_sb[:, cohalf * 512:(cohalf + 1) * 512],
                        start=True, stop=True,
                    )
                    dst = ob[:, cohalf * 512:(cohalf + 1) * 512]
                    if cohalf == 0:
                        nc.vector.tensor_copy(dst, po[:])
                    else:
                        nc.scalar.copy(dst, po[:])
                nc.sync.dma_start(out[tok0:tok0 + P, :], ob[:])

            # filler queue of (kind, fn): independent PE work units — proj
            # steps of the previous group, qkv units + v transposes of the
            # next batch. Popped between attention iterations so the PE
            # never idles; qkv/transpose units are force-drained at batch
            # end (the next batch's attention consumes their outputs), proj
            # units can linger.
            filler = []

            def pop_filler(n=1, reserve=0):
                for _ in range(n):
                    if len(filler) <= reserve:
                        return
                    kind, fn = filler.pop(0)
                    more = fn()
                    if more:
                        filler[0:0] = [("tr", m) for m in more]

            def drain_kinds(kinds):
                rest = []
                while filler:
                    kind, fn = filler.pop(0)
                    if kind in kinds:
                        more = fn()
                        if more:
                            filler[0:0] = [("tr", m) for m in more]
                    else:
                        rest.append((kind, fn))
                filler.extend(rest)

            for b in range(B):
                # qkv work for batch b+1, to be drained during batch b's
                # attention
                if b < B - 1:
                    for tg in (2 * b + 2, 2 * b + 3):
                        for cb in range(3):
                            filler.append(
                                ("qkv", lambda tg=tg, cb=cb: qkv_unit(tg, cb))
                            )

                for qg in range(QG_PER_B):
                    tgq = QG_PER_B * b + qg
                    nkj_total = (qg + 1) * 4
                    OT = [
                        ps_ot.tile([DH + 1, 512], F32, tag="ot", name=f"ot{_h}")
                        for _h in range(H_LOC)
                    ]
                    pending = []  # [(pts, kb, q_lo, idx)] PV delayed 2 blocks

                    def emit_pv(batch):
                        for h in range(H_LOC):
                            for pts, kb_, q_lo_, idx_ in batch:
                                nc.tensor.matmul(
                                    OT[h][:, q_lo_:512],
                                    lhsT=v_aug[
                                        :, kb_, h * (DH + 1):(h + 1) * (DH + 1)
                                    ],
                                    rhs=pts[h][:, q_lo_:512],
                                    start=(idx_ == 0),
                                    stop=(idx_ == nkj_total - 1),
                                )

                    idx = 0
                    for kg in range(qg + 1):
                        diag = kg == qg
                        tgk = QG_PER_B * b + kg
                        for kj in range(4):
                            kb = tgk * 4 + kj
                            q_lo = kj * P if diag else 0
                            pts = []
                            for h in range(H_LOC):
                                hs = slice(h * DH, (h + 1) * DH)
                                st = ps_st.tile(
                                    [P, 512], F32, tag="st", name="st"
                                )
                                nc.tensor.matmul(
                                    st[:, q_lo:512],
                                    lhsT=kT_all[hs, tgk, kj * P:(kj + 1) * P],
                                    rhs=qT_all[hs, tgq, q_lo:512],
                                    start=True, stop=True,
                                )
                                pt = ptp.tile([P, 512], BF16, tag="pt", name="pt")
                                nc.scalar.activation(
                                    pt[:, q_lo:512], st[:, q_lo:512], Exp,
                                    bias=0.0, scale=float(SCALE),
                                )
                                if diag:
                                    nc.vector.tensor_mul(
                                        pt[:, kj * P:(kj + 1) * P],
                                        pt[:, kj * P:(kj + 1) * P],
                                        tri01[:],
                                    )
                                pts.append(pt)
                            pending.append((pts, kb, q_lo, idx))
                            idx += 1
                            if len(pending) == 2 and idx < nkj_total:
                                emit_pv([pending.pop(0)])
                            if idx >= 1:
                                # adaptive pacing: drain fast only when the
                                # queue is deep, so late batches (no qkv
                                # work left) still have proj fillers
                                pop_filler(2 if len(filler) > 9 else 1)
                    emit_pv(pending)
                    if qg == QG_PER_B - 1:
                        # next batch's attention needs qT/kT/v complete
                        drain_kinds(("qkv", "tr"))

                    # epilogue: normalize y = O / den (den in OT row 64)
                    last_group = (b == B - 1) and (qg == QG_PER_B - 1)
                    yT = epp.tile([P, 512], BF16, tag="yt")
                    rbs = []
                    for h in range(H_LOC):
                        den = epp.tile([1, 512], F32, tag="den", name="den")
                        nc.vector.tensor_copy(den[:], OT[h][DH:DH + 1, :])
                        rbr = epp.tile([DH, 512], F32, tag="rbr", name="rbr")
                        nc.gpsimd.partition_broadcast(rbr[:], den[:], channels=DH)
                        rb = epp.tile([DH, 512], F32, tag="rb", name="rb")
                        nc.vector.reciprocal_approx_fast(rb[:], rbr[:])
                        rbs.append(rb)
                        if not last_group:
                            nc.vector.tensor_mul(
                                yT[h * DH:(h + 1) * DH, :], OT[h][0:DH, :],
                                rb[:],
                            )
                    if not last_group:
                        filler += [
                            ("proj",
                             lambda tgq=tgq, yT=yT, ti=ti:
                             proj_step(tgq, yT, ti))
                            for ti in range(4)
                        ]
                    else:
                        # tail: chunk the normalize per token tile and chase
                        # each chunk with its projection matmuls immediately
                        for ti in range(4):
                            ts_ = slice(ti * P, (ti + 1) * P)
                            for h in range(H_LOC):
                                nc.vector.tensor_mul(
                                    yT[h * DH:(h + 1) * DH, ts_],
                                    OT[h][0:DH, ts_], rbs[h][:, ts_],
                                )
                            proj_step(tgq, yT, ti)

            drain_kinds(("qkv", "tr", "proj"))

    nc.compile()
    return nc


def _install_ntff_hook():
    try:
        from antenv.axon_hooks import get_axon_ntff_profile_hook  # noqa: F401
        return
    except ImportError:
        pass
    try:
        import trn_agent_boot.trn_boot as tb
        hook = tb._ntff_profile_via_ctypes("/opt/axon/libaxon_pjrt.so")
        mod = types.ModuleType("antenv.axon_hooks")
        mod.get_axon_ntff_profile_hook = lambda: hook
        mod.set_axon_ntff_profile_hook = lambda h: None
        sys.modules["antenv.axon_hooks"] = mod
    except Exception:
        pass


_NC_CACHE = None
LAST_EXEC_NS = None
LAST_TRACE = None


def kernel(x, w_qkv, w_out, trace=False):
    global _NC_CACHE, LAST_EXEC_NS, LAST_TRACE
    import ml_dtypes
    bf16 = ml_dtypes.bfloat16

    if _NC_CACHE is None:
        _NC_CACHE = build_nc()
    nc = _NC_CACHE

    x = np.asarray(x, dtype=np.float32)
    w_qkv = np.asarray(w_qkv, dtype=np.float32)
    w_out = np.asarray(w_out, dtype=np.float32)

    xT = np.ascontiguousarray(x.reshape(BT, C).T.astype(bf16))  # [C, BT]
    in_maps = []
    for core in range(N_CORES):
        h0 = core * H_LOC * DH  # first local channel
        ch = slice(h0, h0 + H_LOC * DH)
        # wq columns: [q chans | k chans | v chans] for the local heads
        wq_i = np.concatenate(
            [w_qkv[s * C:s * C + C, :][ch, :] for s in range(3)], axis=0
        )  # [384, C]
        in_maps.append({
            "xT": xT,
            "wq": np.ascontiguousarray(wq_i.T.astype(bf16)),        # [C, 384]
            "wo": np.ascontiguousarray(w_out[:, ch].T.astype(bf16)),  # [128, C]
        })

    if trace:
        _install_ntff_hook()
    res = run_bass_kernel_spmd(
        nc, in_maps, core_ids=list(range(N_CORES)), trace=trace
    )
    LAST_EXEC_NS = res.exec_time_ns
    kernel_globals = globals()
    kernel_globals['LAST_RESULT'] = res
    LAST_TRACE = (
        res.instructions_and_trace[1] if res.instructions_and_trace else None
    )

    acc = np.zeros((BT, C), dtype=np.float64)
    for core in range(N_CORES):
        acc += res.results[core]["out"].astype(np.float64)
    return acc.astype(np.float32).reshape(B, T, C)


# revision 51
# speedup vs baseline: 1.1966x; 1.0056x over previous
"""Causal self-attention (B=4, T=1024, C=1024, H=16) on 8 TRN2 NeuronCores.

Sharding: tensor-parallel over heads — 2 heads per core. x is replicated;
each core computes qkv for its heads, attention, and a partial output
projection (its heads' columns of w_out); the host sums the 8 partials.

v2: all-bf16 matmul pipeline + HAM-aware scheduling.
  - All matmul operands are bf16 (PSUM accumulation stays fp32): 1 cyc/row
    at every moving width (fp32r pays 4 cyc/row below 256), halved SBUF
    traffic, and DVE 2x/4x perf modes on the elementwise ops.
  - qkv projection of batch b+1 is interleaved as PE "filler" work into
    the attention phase of batch b, so the PE never idles and the HAM
    clock gate stays at 2.4 GHz (idle gaps re-throttle it to 1.2 GHz).
  - x/w DMAs are chunked per contraction block with per-chunk tiles so the
    first matmul starts ~1us in instead of waiting on whole-tile DMAs.
  - copies are spread across engines: q/k PSUM->SBUF on ACT, v/mask/
    normalize on DVE, proj PSUM->SBUF on GpSimd (otherwise idle).
  - output partials are bf16 (halves output DMA); host sums in fp64.

Per-core dataflow:
  phase 1: qkvT[chan, tok] = wqkvT.T @ xT per 512-token group;
           q,k kept as [d, tok] bf16 (2 heads packed on 128 partitions);
           v PE-transposed to [tok, d] bf16 with a ones column appended.
  phase 2: per (batch, 512-query group, 128-key block):
           ST[key, query] = kT.T @ qT  (2 heads row-packed)
           PT = exp(ST/8) -> bf16  (causal: 512-block skipping; diagonal
           128x128 block gets a multiplicative 0/1 mask on PT)
           OT[d+1, query] += v_aug.T @ PT  (row 64 accumulates the softmax
           denominator via the ones column)
           normalize: y = OT[0:64] * broadcast(1/OT[64]) -> bf16
  phase 3: out[tok, :] = yT.T @ woT, PSUM copied to SBUF bf16, DMA to HBM.
"""

import sys
import types

import numpy as np

import concourse.bacc as bacc
import concourse.mybir as mybir
import concourse.tile as tile
from concourse.bass_utils import run_bass_kernel_spmd
from concourse.masks import make_identity

F32 = mybir.dt.float32
BF16 = mybir.dt.bfloat16
Exp = mybir.ActivationFunctionType.Exp

P = 128
B = 4
T = 1024
C = 1024
N_HEAD = 16
DH = 64
BT = B * T           # 4096 tokens
NCO = C // P         # 8 contraction blocks
NTG = BT // 512      # 8 token groups of 512
QG_PER_B = T // 512  # 2 query groups per batch
N_CORES = 8
H_LOC = N_HEAD // N_CORES  # 2 local heads

SCALE = 1.0 / np.sqrt(np.float32(DH))  # 0.125


def build_nc():
    nc = bacc.Bacc("TRN2", target_bir_lowering=False, debug=False)

    xT = nc.dram_tensor("xT", [C, BT], BF16, kind="ExternalInput")
    wq = nc.dram_tensor("wq", [C, 3 * P], BF16, kind="ExternalInput")
    wo = nc.dram_tensor("wo", [P, C], BF16, kind="ExternalInput")
    out = nc.dram_tensor("out", [BT, C], BF16, kind="ExternalOutput")

    with tile.TileContext(nc) as tc:
        with (
            tc.tile_pool(name="consts", bufs=1) as consts,
            tc.tile_pool(name="vt", bufs=3) as vtp,
            tc.tile_pool(name="pt", bufs=6) as ptp,
            tc.tile_pool(name="ep", bufs=4) as epp,
            tc.tile_pool(name="outp", bufs=4) as outp,
            tc.tile_pool(name="ps_mm", bufs=3, space="PSUM") as ps_mm,
            tc.tile_pool(name="ps_st", bufs=3, space="PSUM") as ps_st,
            tc.tile_pool(name="ps_ot", bufs=2, space="PSUM") as ps_ot,
        ):
            # ---- DMA: per-chunk tiles so the first matmul's deps are just
            # its own two chunk loads. tg0 chunks paired with wq chunks
            # first, then tg1, wo, then the rest.
            # tg0 is chunked per contraction block, issued alternating with
            # wq chunks on SP — the serial ~600ns descriptor-gens naturally
            # stage the arrivals just ahead of the PE's consumption.
            x0_sb = [
                consts.tile([P, 512], BF16, name=f"x0_{co}") for co in range(NCO)
            ]
            xg_sb = [None] + [
                consts.tile([P, NCO, 512], BF16, name=f"xg{tg}")
                for tg in range(1, NTG)
            ]
            wq_sb = [
                consts.tile([P, 3 * P], BF16, name=f"wq{co}") for co in range(NCO)
            ]
            wo_sb = consts.tile([P, C], BF16)

            def x_chunk(tg, co):
                if tg == 0:
                    return x0_sb[co][:]
                return xg_sb[tg][:, co, :]

            def wq_chunk(co):
                return wq_sb[co][:]

            xT_r = xT[:].rearrange("(a p) t -> p a t", p=P)
            for co in range(NCO):
                nc.sync.dma_start(
                    x0_sb[co][:], xT[co * P:(co + 1) * P, 0:512]
                )
                nc.sync.dma_start(wq_sb[co][:], wq[co * P:(co + 1) * P, :])
            nc.sync.dma_start(xg_sb[1][:], xT_r[:, :, 512:1024])
            nc.sync.dma_start(wo_sb[:], wo[:])

            qT_all = consts.tile([P, NTG, 512], BF16)
            kT_all = consts.tile([P, NTG, 512], BF16)
            # both heads' v in one tile: per key block kb the 130 columns are
            # [h0 chans (64) | ones | h1 chans (64) | ones]; head h's PV lhsT
            # is the contiguous 65-column slice starting at h*65. The ones
            # columns accumulate the softmax denominator.
            v_aug = consts.tile([P, BT // P, 2 * (DH + 1)], BF16, name="v")

            ident = consts.tile([P, P], F32)
            make_identity(nc, ident[:])
            # multiplicative causal mask for the diagonal 128x128 block of
            # PT[key, query]: keep where col - row >= 0. Built in f32
            # (affine_select proven there), then cast to bf16.
            tri_f32 = consts.tile([P, P], F32)
            nc.gpsimd.memset(tri_f32[:], 1.0)
            nc.gpsimd.affine_select(
                out=tri_f32[:], in_=tri_f32[:],
                compare_op=mybir.AluOpType.is_ge, fill=0.0,
                base=0, pattern=[[1, P]], channel_multiplier=-1,
            )
            tri01 = consts.tile([P, P], BF16)
            nc.vector.tensor_copy(tri01[:], tri_f32[:])
            ones_f = consts.tile([P, BT // P], BF16)
            nc.vector.memset(ones_f[:], 1.0)
            for h in range(H_LOC):
                nc.vector.tensor_copy(
                    v_aug[:, :, h * (DH + 1) + DH], ones_f[:]
                )

            # ---- qkv projection emission units ----
            # one unit = one (tg, cb) PSUM accumulation group (8 matmuls)
            # plus its PSUM->SBUF copy; v additionally spawns 4 transpose
            # units. Units are either emitted directly (prologue) or pushed
            # through the filler queue during attention.

            def transpose_unit(tg, j, vt):
                kb = tg * 4 + j
                pst = ps_mm.tile([P, P], F32, tag="mm", name="pst")
                nc.tensor.transpose(pst[:], vt[:, j * P:(j + 1) * P], ident[:])
                # both heads in one strided copy: dst groups at 0 and 65
                nc.vector.tensor_copy(
                    v_aug[:, kb, :].rearrange(
                        "p (g c) -> p g c", g=H_LOC
                    )[:, :, 0:DH],
                    pst[:, :].rearrange("p (g c) -> p g c", g=H_LOC),
                )

            def qkv_unit(tg, cb):
                # returns list of follow-up units (v transposes)
                ps = ps_mm.tile([P, 512], F32, tag="mm")
                for co in range(NCO):
                    nc.tensor.matmul(
                        ps[:],
                        lhsT=wq_chunk(co)[:, cb * P:(cb + 1) * P],
                        rhs=x_chunk(tg, co),
                        start=(co == 0), stop=(co == NCO - 1),
                    )
                if cb == 0:
                    nc.scalar.copy(qT_all[:, tg, :], ps[:])
                    return []
                if cb == 1:
                    nc.scalar.copy(kT_all[:, tg, :], ps[:])
                    return []
                vt = vtp.tile([P, 512], F32, tag="vt")
                nc.scalar.copy(vt[:], ps[:])
                return [lambda j=j, vt=vt: transpose_unit(tg, j, vt)
                        for j in range(4)]

            def emit_qkv_group(tg):
                # prologue path: emit everything for one token group now
                followups = []
                for cb in range(3):
                    followups += qkv_unit(tg, cb)
                for f in followups:
                    f()

            # ---- prologue: qkv for tg0, tg1 (feeds attention for b=0) ----
            # later x groups are DMA'd only after the prologue's own loads
            # are in the queues, so they don't starve the first matmuls
            emit_qkv_group(0)
            for tg in (2, 3, 4):
                nc.sync.dma_start(
                    xg_sb[tg][:], xT_r[:, :, tg * 512:(tg + 1) * 512]
                )
            emit_qkv_group(1)
            for tg in (5, 6, 7):
                nc.sync.dma_start(
                    xg_sb[tg][:], xT_r[:, :, tg * 512:(tg + 1) * 512]
                )

            # ---- attention + projection, software-pipelined ----
            # one proj unit covers a full 128-token tile: 2 matmuls into the
            # two 512-column halves, copies to one SBUF tile, one DMA out
            def proj_step(tgq, yT, ti):
                tok0 = tgq * 512 + ti * P
                ob = outp.tile([P, C], BF16, tag="ob", name="ob")
                for cohalf in range(2):
                    po = ps_mm.tile([P, 512], F32, tag="mm", name="po")
                    nc.tensor.matmul(
                        po[:],
                        lhsT=yT[:, ti * P:(ti + 1) * P],
                        rhs=wo_sb[:, cohalf * 512:(cohalf + 1) * 512],
                        start=True, stop=True,
                    )
                    dst = ob[:, cohalf * 512:(cohalf + 1) * 512]
                    if cohalf == 0:
                        nc.vector.tensor_copy(dst, po[:])
                    else:
                        nc.scalar.copy(dst, po[:])
                nc.sync.dma_start(out[tok0:tok0 + P, :], ob[:])

            # filler queue of (kind, fn): independent PE work units — proj
            # steps of the previous group, qkv units + v transposes of the
            # next batch. Popped between attention iterations so the PE
            # never idles; qkv/transpose units are force-drained at batch
            # end (the next batch's attention consumes their outputs), proj
            # units can linger.
            filler = []

            def pop_filler(n=1, reserve=0):
                for _ in range(n):
                    if len(filler) <= reserve:
                        return
                    kind, fn = filler.pop(0)
                    more = fn()
                    if more:
                        filler[0:0] = [("tr", m) for m in more]

            def drain_kinds(kinds):
                rest = []
                while filler:
                    kind, fn = filler.pop(0)
                    if kind in kinds:
                        more = fn()
                        if more:
                            filler[0:0] = [("tr", m) for m in more]
                    else:
                        rest.append((kind, fn))
                filler.extend(rest)

            for b in range(B):
                # qkv work for batch b+1, to be drained during batch b's
                # attention
                if b < B - 1:
                    for tg in (2 * b + 2, 2 * b + 3):
                        for cb in range(3):
                            filler.append(
                                ("qkv", lambda tg=tg, cb=cb: qkv_unit(tg, cb))
                            )

                for qg in range(QG_PER_B):
                    tgq = QG_PER_B * b + qg
                    nkj_total = (qg + 1) * 4
                    OT = [
                        ps_ot.tile([DH + 1, 512], F32, tag="ot", name=f"ot{_h}")
                        for _h in range(H_LOC)
                    ]
                    pending = []  # [(pts, kb, q_lo, idx)] PV delayed 2 blocks

                    def emit_pv(batch):
                        for h in range(H_LOC):
                            for pts, kb_, q_lo_, idx_ in batch:
                                nc.tensor.matmul(
                                    OT[h][:, q_lo_:512],
                                    lhsT=v_aug[
                                        :, kb_, h * (DH + 1):(h + 1) * (DH + 1)
                                    ],
                                    rhs=pts[h][:, q_lo_:512],
                                    start=(idx_ == 0),
                                    stop=(idx_ == nkj_total - 1),
                                )

                    idx = 0
                    for kg in range(qg + 1):
                        diag = kg == qg
                        tgk = QG_PER_B * b + kg
                        for kj in range(4):
                            kb = tgk * 4 + kj
                            q_lo = kj * P if diag else 0
                            pts = []
                            for h in range(H_LOC):
                                hs = slice(h * DH, (h + 1) * DH)
                                st = ps_st.tile(
                                    [P, 512], F32, tag="st", name="st"
                                )
                                nc.tensor.matmul(
                                    st[:, q_lo:512],
                                    lhsT=kT_all[hs, tgk, kj * P:(kj + 1) * P],
                                    rhs=qT_all[hs, tgq, q_lo:512],
                                    start=True, stop=True,
                                )
                                pt = ptp.tile([P, 512], BF16, tag="pt", name="pt")
                                nc.scalar.activation(
                                    pt[:, q_lo:512], st[:, q_lo:512], Exp,
                                    bias=0.0, scale=float(SCALE),
                                )
                                if diag:
                                    nc.vector.tensor_mul(
                                        pt[:, kj * P:(kj + 1) * P],
                                        pt[:, kj * P:(kj + 1) * P],
                                        tri01[:],
                                    )
                                pts.append(pt)
                            pending.append((pts, kb, q_lo, idx))
                            idx += 1
                            if len(pending) == 2 and idx < nkj_total:
                                emit_pv([pending.pop(0)])
                            if idx >= 1:
                                # adaptive pacing: drain fast only when the
                                # queue is deep, so late batches (no qkv
                                # work left) still have proj fillers; in the
                                # final batch hoard a few for the normalize+
                                # projection tail where no new work arrives
                                if b < B - 1:
                                    pop_filler(2 if len(filler) > 9 else 1)
                                elif idx % 2 == 0 and len(filler) > 2:
                                    pop_filler(1)
                    emit_pv(pending)
                    if qg == QG_PER_B - 1:
                        # next batch's attention needs qT/kT/v complete;
                        # at the very end also play the hoarded proj units
                        # so the PE has work during the normalize chain
                        if b == B - 1:
                            drain_kinds(("qkv", "tr", "proj"))
                        else:
                            drain_kinds(("qkv", "tr"))

                    # epilogue: normalize y = O / den (den in OT row 64)
                    last_group = (b == B - 1) and (qg == QG_PER_B - 1)
                    yT = epp.tile([P, 512], BF16, tag="yt")
                    rbs = []
                    for h in range(H_LOC):
                        den = epp.tile([1, 512], F32, tag="den", name="den")
                        nc.vector.tensor_copy(den[:], OT[h][DH:DH + 1, :])
                        rbr = epp.tile([DH, 512], F32, tag="rbr", name="rbr")
                        nc.gpsimd.partition_broadcast(rbr[:], den[:], channels=DH)
                        rb = epp.tile([DH, 512], F32, tag="rb", name="rb")
                        nc.vector.reciprocal_approx_fast(rb[:], rbr[:])
                        rbs.append(rb)
                        if not last_group:
                            nc.vector.tensor_mul(
                                yT[h * DH:(h + 1) * DH, :], OT[h][0:DH, :],
                                rb[:],
                            )
                    if not last_group:
                        filler += [
                            ("proj",
                             lambda tgq=tgq, yT=yT, ti=ti:
                             proj_step(tgq, yT, ti))
                            for ti in range(4)
                        ]
                    else:
                        # tail: chunk the normalize per token tile and chase
                        # each chunk with its projection matmuls immediately
                        for ti in range(4):
                            ts_ = slice(ti * P, (ti + 1) * P)
                            for h in range(H_LOC):
                                nc.vector.tensor_mul(
                                    yT[h * DH:(h + 1) * DH, ts_],
                                    OT[h][0:DH, ts_], rbs[h][:, ts_],
                                )
                            proj_step(tgq, yT, ti)

            drain_kinds(("qkv", "tr", "proj"))

    nc.compile()
    return nc


def _install_ntff_hook():
    try:
        from antenv.axon_hooks import get_axon_ntff_profile_hook  # noqa: F401
        return
    except ImportError:
        pass
    try:
        import trn_agent_boot.trn_boot as tb
        hook = tb._ntff_profile_via_ctypes("/opt/axon/libaxon_pjrt.so")
        mod = types.ModuleType("antenv.axon_hooks")
        mod.get_axon_ntff_profile_hook = lambda: hook
        mod.set_axon_ntff_profile_hook = lambda h: None
        sys.modules["antenv.axon_hooks"] = mod
    except Exception:
        pass


_NC_CACHE = None
LAST_EXEC_NS = None
LAST_TRACE = None


def kernel(x, w_qkv, w_out, trace=False):
    global _NC_CACHE, LAST_EXEC_NS, LAST_TRACE
    import ml_dtypes
    bf16 = ml_dtypes.bfloat16

    if _NC_CACHE is None:
        _NC_CACHE = build_nc()
    nc = _NC_CACHE

    x = np.asarray(x, dtype=np.float32)
    w_qkv = np.asarray(w_qkv, dtype=np.float32)
    w_out = np.asarray(w_out, dtype=np.float32)

    xT = np.ascontiguousarray(x.reshape(BT, C).T.astype(bf16))  # [C, BT]
    in_maps = []
    for core in range(N_CORES):
        h0 = core * H_LOC * DH  # first local channel
        ch = slice(h0, h0 + H_LOC * DH)
        # wq columns: [q chans | k chans | v chans] for the local heads
        wq_i = np.concatenate(
            [w_qkv[s * C:s * C + C, :][ch, :] for s in range(3)], axis=0
        )  # [384, C]
        in_maps.append({
            "xT": xT,
            "wq": np.ascontiguousarray(wq_i.T.astype(bf16)),        # [C, 384]
            "wo": np.ascontiguousarray(w_out[:, ch].T.astype(bf16)),  # [128, C]
        })

    if trace:
        _install_ntff_hook()
    res = run_bass_kernel_spmd(
        nc, in_maps, core_ids=list(range(N_CORES)), trace=trace
    )
    LAST_EXEC_NS = res.exec_time_ns
    kernel_globals = globals()
    kernel_globals['LAST_RESULT'] = res
    LAST_TRACE = (
        res.instructions_and_trace[1] if res.instructions_and_trace else None
    )

    acc = np.zeros((BT, C), dtype=np.float64)
    for core in range(N_CORES):
        acc += res.results[core]["out"].astype(np.float64)
    return acc.astype(np.float32).reshape(B, T, C)
